# revision 1
# baseline (speedup 1.0000x reference)
"""BiLSTM (2-layer, masked/ragged) Trainium2 kernel.

Sharding: 8 cores = 2 directions x 4 batch shards (16 each). Backward
direction cores receive time-reversed inputs from the host, so the device
program is direction-agnostic SPMD. Layer-0 direction outputs are swapped
between fwd/bwd partner cores with an 8-core AllGather of time-reversed
copies; each core then computes layer-1 input projections from its own +
partner halves and runs the layer-1 scan.

All matmuls in bf16 (weights stationary, gates on PSUM partitions, batch on
the free dim), cell state and elementwise chain in fp32.
"""

import os
import numpy as np
import ml_dtypes

import concourse.bass as bass
import concourse.bacc as bacc
import concourse.mybir as mybir
import concourse.tile as tile
from concourse import bass_utils

bf16 = ml_dtypes.bfloat16
f32 = mybir.dt.float32
bf = mybir.dt.bfloat16

T, B, D, H = 512, 64, 512, 512
NCORES = 8
BS = B // 4  # 16, batch shard per core
G = 4 * H  # 2048 gates
GT = G // 128  # 16 gate tiles
KH = H // 128  # 4 k-chunks for hidden contraction
TC = 32  # timesteps per phase chunk
NC_CHUNKS = T // TC  # 16

_compiled = {}

# gate-column permutation: store gate tile g (type q=g//4 in i,f,g,o; hidden
# tile j=g%4) at column block perm(g) so the layout is [i,f,o,g] and one
# sigmoid covers i,f,o contiguously.
_PERM = [(q, j) for q in range(4) for j in range(4)]
def _perm(g):
    q, j = g // 4, g % 4
    return {0: j, 1: 4 + j, 2: 12 + j, 3: 8 + j}[q]


def _build(t_steps=T, dbg=False):
    nc = bacc.Bacc("TRN2", target_bir_lowering=False, debug=False,
                   num_devices=NCORES)
    nchunks = t_steps // TC

    # ---- per-core external inputs ----
    xT = nc.dram_tensor("xT", (D, t_steps, BS), bf, kind="ExternalInput")
    maskd = nc.dram_tensor("maskd", (t_steps, KH * BS), bf, kind="ExternalInput")
    whh0T = nc.dram_tensor("whh0T", (KH, 128, G), bf, kind="ExternalInput")
    wih0T = nc.dram_tensor("wih0T", (KH, 128, G), bf, kind="ExternalInput")
    whh1T = nc.dram_tensor("whh1T", (KH, 128, G), bf, kind="ExternalInput")
    wih1oT = nc.dram_tensor("wih1oT", (KH, 128, G), bf, kind="ExternalInput")
    wih1pT = nc.dram_tensor("wih1pT", (KH, 128, G), bf, kind="ExternalInput")
    b0c = nc.dram_tensor("b0c", (GT, 128), f32, kind="ExternalInput")
    b1c = nc.dram_tensor("b1c", (GT, 128), f32, kind="ExternalInput")
    y1 = nc.dram_tensor("y1", (t_steps, 128, KH * BS), f32, kind="ExternalOutput")
    if dbg:
        xwb0o = nc.dram_tensor("xwb0o", (t_steps, 128, GT * BS), bf, kind="ExternalOutput")
        xwb1o = nc.dram_tensor("xwb1o", (t_steps, 128, GT * BS), bf, kind="ExternalOutput")
        y0o = nc.dram_tensor("y0o", (t_steps, 128, KH * BS), bf, kind="ExternalOutput")
        ago = nc.dram_tensor("ago", (NCORES * t_steps, 128, KH * BS), bf, kind="ExternalOutput")

    with tile.TileContext(nc) as tc:
        with (
            tc.tile_pool(name="wpool", bufs=1) as wpool,
            tc.tile_pool(name="xpool", bufs=3) as xpool,
            tc.tile_pool(name="gpool", bufs=3) as gpool,
            tc.tile_pool(name="spool", bufs=3) as spool,
            tc.tile_pool(name="state", bufs=1) as state,
            tc.tile_pool(name="psA", bufs=2, space="PSUM") as psA,
            tc.tile_pool(name="psS", bufs=2, space="PSUM") as psS,
            tc.tile_pool(name="dram", bufs=1, space="DRAM") as dram,
        ):
            # ---- internal DRAM ----
            xwb0 = dram.tile([t_steps, 128, GT * BS], bf)
            xwb1 = dram.tile([t_steps, 128, GT * BS], bf)
            y0 = dram.tile([t_steps, 128, KH * BS], bf)
            y0x = dram.tile([t_steps, 128, KH * BS], bf)
            ag = dram.tile([NCORES * t_steps, 128, KH * BS], bf)

            # ---- resident weights ----
            def load_w(name, src):
                t = wpool.tile([128, KH * G], bf, tag=name)
                for k in range(KH):
                    nc.sync.dma_start(t[:, k * G:(k + 1) * G], src.ap()[k])
                return t

            whh0_sb = load_w("whh0", whh0T)
            wih0_sb = load_w("wih0", wih0T)
            whh1_sb = load_w("whh1", whh1T)
            wih1o_sb = load_w("wih1o", wih1oT)
            wih1p_sb = load_w("wih1p", wih1pT)
            bias_sb = wpool.tile([128, 2 * GT], f32, tag="bias")
            nc.sync.dma_start(bias_sb[:, 0:GT], b0c.ap().transpose([1, 0]))
            nc.sync.dma_start(bias_sb[:, GT:2 * GT], b1c.ap().transpose([1, 0]))

            # ---- phase A / D: input projections -> xwb dram ----
            def proj(dst, w_sbs, srcs, bias_col, dbg_dst=None):
                """dst[t,p,g*BS+b] = sum_j srcs[j] @ w_sbs-chunks + bias."""
                nk = len(w_sbs) * KH
                for ncnk in range(nchunks):
                    t0 = ncnk * TC
                    rhs = xpool.tile([128, nk, TC, BS], bf, tag="projx")
                    ji = 0
                    for w_sb, src in zip(w_sbs, srcs):
                        for k in range(KH):
                            nc.sync.dma_start(rhs[:, ji], src(k, t0))
                            ji += 1
                    for g in range(GT):
                        ps = psA.tile([128, TC * BS], f32, tag="psA")
                        ji = 0
                        for w_sb in w_sbs:
                            for k in range(KH):
                                nc.tensor.matmul(
                                    ps[:],
                                    w_sb[:, k * G + g * 128: k * G + (g + 1) * 128],
                                    rhs[:, ji],
                                    start=(ji == 0),
                                    stop=(ji == nk - 1),
                                )
                                ji += 1
                        g_sb = gpool.tile([128, TC * BS], bf, tag="projg")
                        nc.scalar.activation(
                            g_sb[:], ps[:], mybir.ActivationFunctionType.Identity,
                            bias=bias_sb[:, bias_col + g: bias_col + g + 1],
                        )
                        P = _perm(g)
                        nc.sync.dma_start(
                            dst[t0:t0 + TC, :, P * BS:(P + 1) * BS].transpose([1, 0, 2]),
                            g_sb[:].rearrange("p (t b) -> p t b", t=TC),
                        )
                        if dbg_dst is not None:
                            nc.sync.dma_start(
                                dbg_dst.ap()[t0:t0 + TC, :, g * BS:(g + 1) * BS].transpose([1, 0, 2]),
                                g_sb[:].rearrange("p (t b) -> p t b", t=TC),
                            )

            proj(
                xwb0, [wih0_sb],
                [lambda k, t0: xT.ap()[k * 128:(k + 1) * 128, t0:t0 + TC, :]],
                bias_col=0,
                dbg_dst=xwb0o if dbg else None,
            )

            # ---- scan helper ----
            def scan(xwb, whh_sb, y_dst, yx_dst, out_f32, dbg_dst=None):
                h2f = state.tile([128, KH * BS], f32, tag="h2f")
                cst = state.tile([128, KH * BS], f32, tag="cst")
                nc.gpsimd.memset(h2f[:], 0.0)
                nc.gpsimd.memset(cst[:], 0.0)
                for t in range(t_steps):
                    m_sb = spool.tile([128, KH * BS], bf, tag="m")
                    nc.sync.dma_start(
                        m_sb[:], maskd.ap()[t:t + 1, :].broadcast_to([128, KH * BS]))
                    xw_sb = spool.tile([128, GT * BS], bf, tag="xw")
                    nc.sync.dma_start(xw_sb[:], xwb[t])
                    h_in = spool.tile([128, KH * BS], bf, tag="hin")
                    nc.vector.tensor_mul(h_in[:], h2f[:], m_sb[:])
                    nc.vector.tensor_mul(cst[:], cst[:], m_sb[:])
                    ps = psS.tile([128, GT * BS], f32, tag="psS")
                    for g in range(GT):
                        P = _perm(g)
                        for k in range(KH):
                            nc.tensor.matmul(
                                ps[:, P * BS:(P + 1) * BS],
                                whh_sb[:, k * G + g * 128: k * G + (g + 1) * 128],
                                h_in[:, k * BS:(k + 1) * BS],
                                start=(k == 0),
                                stop=(k == KH - 1),
                            )
                    nc.vector.tensor_add(ps[:], ps[:], xw_sb[:])
                    # permuted gate cols: [0:H') i | [H':2H') f | [2H':3H') o
                    # | [3H':4H') g-candidate  (H'=KH*BS)
                    HB = KH * BS
                    sif = spool.tile([128, 3 * HB], f32, tag="sif")
                    nc.scalar.activation(
                        sif[:], ps[:, 0:3 * HB], mybir.ActivationFunctionType.Sigmoid)
                    tng = spool.tile([128, HB], f32, tag="tng")
                    nc.scalar.activation(
                        tng[:], ps[:, 3 * HB:4 * HB], mybir.ActivationFunctionType.Tanh)
                    ig = spool.tile([128, HB], f32, tag="ig")
                    nc.vector.tensor_mul(ig[:], sif[:, 0:HB], tng[:])
                    fc = spool.tile([128, HB], f32, tag="fc")
                    nc.vector.tensor_mul(fc[:], sif[:, HB:2 * HB], cst[:])
                    nc.vector.tensor_add(cst[:], fc[:], ig[:])
                    tc2 = spool.tile([128, HB], f32, tag="tc2")
                    nc.scalar.activation(
                        tc2[:], cst[:], mybir.ActivationFunctionType.Tanh)
                    nc.vector.tensor_mul(h2f[:], sif[:, 2 * HB:3 * HB], tc2[:])
                    y_sb = spool.tile([128, HB], f32 if out_f32 else bf, tag="y")
                    nc.vector.tensor_mul(y_sb[:], h2f[:], m_sb[:])
                    nc.sync.dma_start(y_dst[t], y_sb[:])
                    if yx_dst is not None:
                        nc.sync.dma_start(yx_dst[t_steps - 1 - t], y_sb[:])
                    if dbg_dst is not None:
                        nc.sync.dma_start(dbg_dst.ap()[t], y_sb[:])

            scan(xwb0, whh0_sb, y0, y0x, out_f32=False,
                 dbg_dst=y0o if dbg else None)

            # ---- exchange ----
            nc.gpsimd.collective_compute(
                "AllGather", mybir.AluOpType.bypass,
                ins=[y0x.opt()], outs=[ag.opt()],
                replica_groups=[list(range(NCORES))],
            )
            partner_row = nc.snap(((nc.partition_id() + 4) % 8) * t_steps)

            if dbg:
                nc.sync.dma_start(ago.ap()[:], ag[:])

            proj(
                xwb1, [wih1o_sb, wih1p_sb],
                [
                    lambda k, t0: y0[t0:t0 + TC, :, k * BS:(k + 1) * BS].transpose([1, 0, 2]),
                    lambda k, t0: ag[bass.ds(partner_row + t0, TC), :, k * BS:(k + 1) * BS].transpose([1, 0, 2]),
                ],
                bias_col=GT,
                dbg_dst=xwb1o if dbg else None,
            )

            scan(xwb1, whh1_sb, y1.ap(), None, out_f32=True)

    nc.compile()
    return nc


def _prep_inputs(x, lengths, weights, t_steps=T):
    """Build the 8 per-core input maps."""
    active = (np.arange(T)[:, None] < np.asarray(lengths)[None, :]).astype(np.float32)
    in_maps = []
    for c in range(NCORES):
        d, s = c // 4, c % 4
        bsl = slice(s * BS, (s + 1) * BS)
        pre = "f" if d == 0 else "b"
        xs = np.asarray(x[:, bsl, :], np.float32)
        am = active[:, bsl]
        if d == 1:
            xs = xs[::-1]
            am = am[::-1]
        xs = xs[:t_steps]
        am = am[:t_steps]

        W_ih0 = np.asarray(weights[f"{pre}W_ih0"], np.float32)
        W_hh0 = np.asarray(weights[f"{pre}W_hh0"], np.float32)
        W_ih1 = np.asarray(weights[f"{pre}W_ih1"], np.float32)
        W_hh1 = np.asarray(weights[f"{pre}W_hh1"], np.float32)
        b0 = np.asarray(weights[f"{pre}b0"], np.float32)
        b1 = np.asarray(weights[f"{pre}b1"], np.float32)
        own = W_ih1[:, :512] if d == 0 else W_ih1[:, 512:]
        par = W_ih1[:, 512:] if d == 0 else W_ih1[:, :512]

        in_maps.append({
            "xT": np.ascontiguousarray(xs.transpose(2, 0, 1)).astype(bf16),
            "maskd": np.ascontiguousarray(np.tile(am, (1, KH))).astype(bf16),
            "whh0T": np.ascontiguousarray(W_hh0.T.reshape(KH, 128, G)).astype(bf16),
            "wih0T": np.ascontiguousarray(W_ih0.T.reshape(KH, 128, G)).astype(bf16),
            "whh1T": np.ascontiguousarray(W_hh1.T.reshape(KH, 128, G)).astype(bf16),
            "wih1oT": np.ascontiguousarray(own.T.reshape(KH, 128, G)).astype(bf16),
            "wih1pT": np.ascontiguousarray(par.T.reshape(KH, 128, G)).astype(bf16),
            "b0c": np.ascontiguousarray(b0.reshape(GT, 128)).astype(np.float32),
            "b1c": np.ascontiguousarray(b1.reshape(GT, 128)).astype(np.float32),
        })
    return in_maps


def _assemble(results, t_steps=T):
    out = np.zeros((t_steps, B, 2 * H), np.float32)
    for c in range(NCORES):
        d, s = c // 4, c % 4
        arr = results[c]["y1"].reshape(t_steps, 128, KH, BS)
        if d == 1:
            arr = arr[::-1]
        # [t, p, j, b] -> [t, b, j*128+p]
        blk = arr.transpose(0, 3, 2, 1).reshape(t_steps, BS, H)
        out[:, s * BS:(s + 1) * BS, d * H:(d + 1) * H] = blk
    return out


def kernel(x, lengths, fW_ih0, fW_hh0, fb0, bW_ih0, bW_hh0, bb0,
           fW_ih1, fW_hh1, fb1, bW_ih1, bW_hh1, bb1, _t_steps=T,
           _want_trace=False, _dbg=False):
    weights = dict(fW_ih0=fW_ih0, fW_hh0=fW_hh0, fb0=fb0,
                   bW_ih0=bW_ih0, bW_hh0=bW_hh0, bb0=bb0,
                   fW_ih1=fW_ih1, fW_hh1=fW_hh1, fb1=fb1,
                   bW_ih1=bW_ih1, bW_hh1=bW_hh1, bb1=bb1)
    key = (_t_steps, _dbg)
    if key not in _compiled:
        _compiled[key] = _build(_t_steps, dbg=_dbg)
    nc = _compiled[key]
    in_maps = _prep_inputs(x, lengths, weights, _t_steps)
    res = bass_utils.run_bass_kernel_spmd(
        nc, in_maps, core_ids=list(range(NCORES)), trace=_want_trace)
    out = _assemble(res.results, _t_steps)
    if _want_trace or _dbg:
        kernel.last_results = res
    return out



# revision 8
# speedup vs baseline: 1.1221x; 1.1221x over previous
"""BiLSTM (2-layer, masked/ragged) Trainium2 kernel.

Sharding: 8 cores = 2 directions x 4 batch shards (16 each). Backward cores
receive time-reversed inputs from the host, so the device program is
direction-agnostic SPMD. Layer-0 outputs are exchanged between fwd/bwd
partner cores with pairwise AllGathers; each core computes layer-1 input
projections from its own half plus a time-reversed read of the partner half.

Scan step structure (per timestep, PSUM bank-parallel):
  - xw (precomputed input projection + bias) is injected into the four
    per-gate PSUM accumulators with identity matmuls (no DVE add).
  - 64 Whh matmuls (free dim 16) accumulate over it, ordered g,i,f,o so the
    sigmoid/tanh chain for c_t overlaps the tail of the PE stream.
  - Activations split per gate; h_{t+1}'s masked state is produced as
    (sigmoid(o)*mask) * tanh(c) so only one DVE op trails the last tanh.
All matmuls bf16 (weights stationary, gates on PSUM partitions, batch on the
free dim); cell state and elementwise chain fp32. DMAs batched 8 steps.
"""

import numpy as np
import ml_dtypes

import concourse.bass as bass
import concourse.bacc as bacc
import concourse.mybir as mybir
import concourse.tile as tile
from concourse import bass_utils

bf16 = ml_dtypes.bfloat16
f32 = mybir.dt.float32
bf = mybir.dt.bfloat16

T, B, D, H = 512, 64, 512, 512
NCORES = 8
BS = B // 4          # 16 batch per core
G = 4 * H            # 2048 gates
GT = G // 128        # 16 gate tiles
KH = H // 128        # 4 k-chunks
HB = KH * BS         # 64 state cols
SC = 8               # scan steps per DMA batch
TC = 32              # proj timesteps per chunk
NCH = T // TC

SIG = mybir.ActivationFunctionType.Sigmoid
TANH = mybir.ActivationFunctionType.Tanh
IDENT = mybir.ActivationFunctionType.Identity

_compiled = {}


def _build(t_steps=T, dbg=False, nocc=False):
    nc = bacc.Bacc("TRN2", target_bir_lowering=False, debug=False,
                   num_devices=NCORES)
    nchunks = t_steps // TC

    xT = nc.dram_tensor("xT", (D, t_steps, BS), bf, kind="ExternalInput")
    maskd = nc.dram_tensor("maskd", (t_steps + 1, 128, HB), bf, kind="ExternalInput")
    whh0T = nc.dram_tensor("whh0T", (KH, 128, G), bf, kind="ExternalInput")
    wih0T = nc.dram_tensor("wih0T", (KH, 128, G), bf, kind="ExternalInput")
    whh1T = nc.dram_tensor("whh1T", (KH, 128, G), bf, kind="ExternalInput")
    wih1oT = nc.dram_tensor("wih1oT", (KH, 128, G), bf, kind="ExternalInput")
    wih1pT = nc.dram_tensor("wih1pT", (KH, 128, G), bf, kind="ExternalInput")
    identT = nc.dram_tensor("identT", (128, 128), bf, kind="ExternalInput")
    b0c = nc.dram_tensor("b0c", (GT, 128), f32, kind="ExternalInput")
    b1c = nc.dram_tensor("b1c", (GT, 128), f32, kind="ExternalInput")
    y1 = nc.dram_tensor("y1", (t_steps, 128, HB), bf, kind="ExternalOutput")
    if dbg:
        xwb0o = nc.dram_tensor("xwb0o", (t_steps, 128, GT * BS), bf, kind="ExternalOutput")
        y0o = nc.dram_tensor("y0o", (t_steps, 128, HB), bf, kind="ExternalOutput")
        xwb1o = nc.dram_tensor("xwb1o", (t_steps, 128, GT * BS), bf, kind="ExternalOutput")
    with tile.TileContext(nc) as tc:
        with (
            tc.tile_pool(name="wpool", bufs=1) as wpool,
            tc.tile_pool(name="xpool", bufs=3) as xpool,
            tc.tile_pool(name="gpool", bufs=3) as gpool,
            tc.tile_pool(name="spool", bufs=3) as spool,
            tc.tile_pool(name="opool", bufs=2) as opool,
            tc.tile_pool(name="mpool", bufs=2) as mpool,
            tc.tile_pool(name="state", bufs=1) as state,
            tc.tile_pool(name="psA", bufs=2, space="PSUM") as psA,
            tc.tile_pool(name="psS", bufs=1, space="PSUM") as psS,
            tc.tile_pool(name="dram", bufs=1, space="DRAM") as dram,
        ):
            xwb0 = dram.tile([t_steps, 128, GT * BS], bf)
            xwb1 = dram.tile([t_steps, 128, GT * BS], bf)
            y0 = dram.tile([t_steps, 128, HB], bf)
            ag = dram.tile([2 * t_steps, 128, HB], bf)

            def load_w(name, src):
                t = wpool.tile([128, KH * G], bf, tag=name)
                for k in range(KH):
                    nc.sync.dma_start(t[:, k * G:(k + 1) * G], src.ap()[k])
                return t

            whh0_sb = load_w("whh0", whh0T)
            wih0_sb = load_w("wih0", wih0T)
            whh1_sb = load_w("whh1", whh1T)
            wih1o_sb = load_w("wih1o", wih1oT)
            wih1p_sb = load_w("wih1p", wih1pT)
            ident_sb = wpool.tile([128, 128], bf, tag="ident")
            nc.sync.dma_start(ident_sb[:], identT.ap())
            bias_sb = wpool.tile([128, 2 * GT], f32, tag="bias")
            nc.sync.dma_start(bias_sb[:, 0:GT], b0c.ap().transpose([1, 0]))
            nc.sync.dma_start(bias_sb[:, GT:2 * GT], b1c.ap().transpose([1, 0]))

            # ---- input projections -> xwb dram ----
            def proj(dst, w_sbs, srcs, bias_col):
                nk = len(w_sbs) * KH
                for ch in range(nchunks):
                    t0 = ch * TC
                    rhs = xpool.tile([128, nk, TC, BS], bf, tag="projx")
                    ji = 0
                    for w_sb, src in zip(w_sbs, srcs):
                        for k in range(KH):
                            nc.sync.dma_start(rhs[:, ji], src(k, t0))
                            ji += 1
                    for g in range(GT):
                        ps = psA.tile([128, TC * BS], f32, tag="psA")
                        ji = 0
                        for w_sb in w_sbs:
                            for k in range(KH):
                                nc.tensor.matmul(
                                    ps[:],
                                    w_sb[:, k * G + g * 128: k * G + (g + 1) * 128],
                                    rhs[:, ji],
                                    start=(ji == 0),
                                    stop=(ji == nk - 1),
                                )
                                ji += 1
                        g_sb = gpool.tile([128, TC * BS], bf, tag="projg")
                        nc.scalar.activation(
                            g_sb[:], ps[:], IDENT,
                            bias=bias_sb[:, bias_col + g: bias_col + g + 1],
                        )
                        nc.sync.dma_start(
                            dst[t0:t0 + TC, :, g * BS:(g + 1) * BS].transpose([1, 0, 2]),
                            g_sb[:].rearrange("p (t b) -> p t b", t=TC),
                        )

            proj(
                xwb0, [wih0_sb],
                [lambda k, t0: xT.ap()[k * 128:(k + 1) * 128, t0:t0 + TC, :]],
                bias_col=0,
            )
            if dbg:
                nc.sync.dma_start(xwb0o.ap()[:], xwb0[:])

            # ---- recurrent scan ----
            # natural gate-type order in xw columns / weight columns: i,f,g,o
            QI = {"i": 0, "f": 1, "g": 2, "o": 3}

            def scan(xwb, whh_sb, y_dst):
                h0 = state.tile([128, HB], bf, tag="h0")
                cst = state.tile([128, HB], f32, tag="cst")
                nc.gpsimd.memset(h0[:], 0.0)
                nc.gpsimd.memset(cst[:], 0.0)
                hprev = h0[:]
                xw = mk = ob = None
                for t in range(t_steps):
                    j = t % SC
                    if j == 0:
                        xw = xpool.tile([128, SC, GT * BS], bf, tag="scanx")
                        nc.sync.dma_start(xw[:], xwb[t:t + SC].transpose([1, 0, 2]))
                        mk = mpool.tile([128, SC + 1, HB], bf, tag="mk")
                        nc.sync.dma_start(
                            mk[:], maskd.ap()[t:t + SC + 1].transpose([1, 0, 2]))
                        ob = opool.tile([128, SC, HB], bf, tag="ob")
                    # c state mask (c *= m[t]); h mask was folded into hprev
                    if t > 0:
                        nc.vector.tensor_mul(cst[:], cst[:], mk[:, j])
                    ps = {q: psS.tile([128, 512], f32, tag=f"ps{q}",
                                      name=f"ps{q}") for q in "ifgo"}
                    for q in "ifgo":
                        nc.tensor.matmul(
                            ps[q][:, 0:HB], ident_sb[:],
                            xw[:, j, QI[q] * HB:(QI[q] + 1) * HB],
                            start=True, stop=False, skip_group_check=True)
                    for q in "gifo":
                        qi = QI[q]
                        for gt in range(4):
                            for k in range(KH):
                                nc.tensor.matmul(
                                    ps[q][:, gt * BS:(gt + 1) * BS],
                                    whh_sb[:, k * G + qi * 512 + gt * 128:
                                           k * G + qi * 512 + (gt + 1) * 128],
                                    hprev[:, k * BS:(k + 1) * BS],
                                    start=False, stop=(k == KH - 1),
                                    skip_group_check=True)
                    tg = spool.tile([128, HB], f32, tag="tg")
                    nc.scalar.activation(tg[:], ps["g"][:, 0:HB], TANH)
                    si = spool.tile([128, HB], f32, tag="si")
                    nc.scalar.activation(si[:], ps["i"][:, 0:HB], SIG)
                    ig = spool.tile([128, HB], f32, tag="ig")
                    nc.vector.tensor_mul(ig[:], si[:], tg[:])
                    sf = spool.tile([128, HB], f32, tag="sf")
                    nc.scalar.activation(sf[:], ps["f"][:, 0:HB], SIG)
                    fc = spool.tile([128, HB], f32, tag="fc")
                    nc.vector.tensor_mul(fc[:], sf[:], cst[:])
                    nc.vector.tensor_add(cst[:], fc[:], ig[:])
                    so = spool.tile([128, HB], f32, tag="so")
                    nc.scalar.activation(so[:], ps["o"][:, 0:HB], SIG)
                    tc2 = spool.tile([128, HB], f32, tag="tc2")
                    nc.scalar.activation(tc2[:], cst[:], TANH)
                    if t + 1 < t_steps:
                        # next-state path first: hm = (so*m[t+1]) * tanh(c)
                        som = spool.tile([128, HB], f32, tag="som")
                        nc.vector.tensor_mul(som[:], so[:], mk[:, j + 1])
                        hm = spool.tile([128, HB], bf, tag="hm")
                        nc.vector.tensor_mul(hm[:], som[:], tc2[:])
                        hprev = hm[:]
                    h2 = spool.tile([128, HB], f32, tag="h2")
                    nc.vector.tensor_mul(h2[:], so[:], tc2[:])
                    nc.vector.tensor_mul(ob[:, j], h2[:], mk[:, j])
                    if j == SC - 1:
                        nc.sync.dma_start(
                            y_dst[t - SC + 1:t + 1].transpose([1, 0, 2]), ob[:])

            scan(xwb0, whh0_sb, y0)
            if dbg:
                nc.sync.dma_start(y0o.ap()[:], y0[:])

            # ---- exchange (pairwise fwd<->bwd) ----
            if nocc:
                nc.sync.dma_start(ag[0:t_steps], y0[:])
                nc.sync.dma_start(ag[t_steps:2 * t_steps], y0[:])
                partner_row = nc.snap(t_steps)
            else:
                nc.gpsimd.collective_compute(
                    "AllGather", mybir.AluOpType.bypass,
                    ins=[y0.opt()], outs=[ag.opt()],
                    replica_groups=[[0, 4], [1, 5], [2, 6], [3, 7]],
                )
                partner_row = nc.snap(
                    ((nc.partition_id() // 4 + 1) % 2) * t_steps)

            def par_src(k, t0):
                # partner rows, time-reversed: own step tau needs partner row
                # (T-1-tau); rows [T-TC-t0, T-t0) reversed.
                return (ag[bass.ds(partner_row + (t_steps - TC - t0), TC)]
                        [::-1, :, k * BS:(k + 1) * BS].transpose([1, 0, 2]))

            proj(
                xwb1, [wih1o_sb, wih1p_sb],
                [
                    lambda k, t0: y0[t0:t0 + TC, :, k * BS:(k + 1) * BS].transpose([1, 0, 2]),
                    par_src,
                ],
                bias_col=GT,
            )
            if dbg:
                nc.sync.dma_start(xwb1o.ap()[:], xwb1[:])

            scan(xwb1, whh1_sb, y1.ap())

    nc.compile()
    return nc


def _prep_inputs(x, lengths, weights, t_steps=T):
    active = (np.arange(T)[:, None] < np.asarray(lengths)[None, :]).astype(np.float32)
    ident = np.eye(128, dtype=bf16)
    in_maps = []
    for c in range(NCORES):
        d, s = c // 4, c % 4
        bsl = slice(s * BS, (s + 1) * BS)
        pre = "f" if d == 0 else "b"
        xs = np.asarray(x[:, bsl, :], np.float32)
        am = active[:, bsl]
        if d == 1:
            xs = xs[::-1]
            am = am[::-1]
        xs = xs[:t_steps]
        am = am[:t_steps]

        W_ih0 = np.asarray(weights[f"{pre}W_ih0"], np.float32)
        W_hh0 = np.asarray(weights[f"{pre}W_hh0"], np.float32)
        W_ih1 = np.asarray(weights[f"{pre}W_ih1"], np.float32)
        W_hh1 = np.asarray(weights[f"{pre}W_hh1"], np.float32)
        b0 = np.asarray(weights[f"{pre}b0"], np.float32)
        b1 = np.asarray(weights[f"{pre}b1"], np.float32)
        own = W_ih1[:, :512] if d == 0 else W_ih1[:, 512:]
        par = W_ih1[:, 512:] if d == 0 else W_ih1[:, :512]

        amk = np.tile(am, (1, KH)).astype(bf16)          # [T, HB]
        mfull = np.zeros((t_steps + 1, 128, HB), bf16)
        mfull[:t_steps] = amk[:, None, :]

        in_maps.append({
            "xT": np.ascontiguousarray(xs.transpose(2, 0, 1)).astype(bf16),
            "maskd": mfull,
            "whh0T": np.ascontiguousarray(W_hh0.T.reshape(KH, 128, G)).astype(bf16),
            "wih0T": np.ascontiguousarray(W_ih0.T.reshape(KH, 128, G)).astype(bf16),
            "whh1T": np.ascontiguousarray(W_hh1.T.reshape(KH, 128, G)).astype(bf16),
            "wih1oT": np.ascontiguousarray(own.T.reshape(KH, 128, G)).astype(bf16),
            "wih1pT": np.ascontiguousarray(par.T.reshape(KH, 128, G)).astype(bf16),
            "identT": ident,
            "b0c": np.ascontiguousarray(b0.reshape(GT, 128)).astype(np.float32),
            "b1c": np.ascontiguousarray(b1.reshape(GT, 128)).astype(np.float32),
        })
    return in_maps


def _assemble(results, t_steps=T):
    out = np.zeros((t_steps, B, 2 * H), np.float32)
    for c in range(NCORES):
        d, s = c // 4, c % 4
        arr = results[c]["y1"].astype(np.float32).reshape(t_steps, 128, KH, BS)
        if d == 1:
            arr = arr[::-1]
        blk = arr.transpose(0, 3, 2, 1).reshape(t_steps, BS, H)
        out[:, s * BS:(s + 1) * BS, d * H:(d + 1) * H] = blk
    return out


def kernel(x, lengths, fW_ih0, fW_hh0, fb0, bW_ih0, bW_hh0, bb0,
           fW_ih1, fW_hh1, fb1, bW_ih1, bW_hh1, bb1, _t_steps=T,
           _want_trace=False, _dbg=False):
    weights = dict(fW_ih0=fW_ih0, fW_hh0=fW_hh0, fb0=fb0,
                   bW_ih0=bW_ih0, bW_hh0=bW_hh0, bb0=bb0,
                   fW_ih1=fW_ih1, fW_hh1=fW_hh1, fb1=fb1,
                   bW_ih1=bW_ih1, bW_hh1=bW_hh1, bb1=bb1)
    key = (_t_steps, _dbg)
    if key not in _compiled:
        _compiled[key] = _build(_t_steps, dbg=_dbg)
    nc = _compiled[key]
    in_maps = _prep_inputs(x, lengths, weights, _t_steps)
    res = bass_utils.run_bass_kernel_spmd(
        nc, in_maps, core_ids=list(range(NCORES)), trace=_want_trace)
    out = _assemble(res.results, _t_steps)
    if _want_trace or _dbg:
        kernel.last_results = res
    return out


# revision 13
# speedup vs baseline: 1.1304x; 1.0074x over previous
"""BiLSTM (2-layer, masked/ragged) Trainium2 kernel.

Sharding: 8 cores = 2 directions x 4 batch shards (16 each). Backward cores
receive time-reversed inputs from the host, so the device program is
direction-agnostic SPMD. Layer-0 outputs are exchanged between fwd/bwd
partner cores with pairwise AllGathers; each core computes layer-1 input
projections from its own half plus a time-reversed read of the partner half.

Scan step structure (per timestep, PSUM bank-parallel):
  - xw (precomputed input projection + bias) is injected into the four
    per-gate PSUM accumulators with identity matmuls (no DVE add).
  - 64 Whh matmuls (free dim 16) accumulate over it, ordered g,i,f,o so the
    sigmoid/tanh chain for c_t overlaps the tail of the PE stream.
  - Activations split per gate; h_{t+1}'s masked state is produced as
    (sigmoid(o)*mask) * tanh(c) so only one DVE op trails the last tanh.
All matmuls bf16 (weights stationary, gates on PSUM partitions, batch on the
free dim); cell state and elementwise chain fp32. DMAs batched 8 steps.
"""

import numpy as np
import ml_dtypes

import concourse.bass as bass
import concourse.bacc as bacc
import concourse.mybir as mybir
import concourse.tile as tile
from concourse import bass_utils

bf16 = ml_dtypes.bfloat16
f32 = mybir.dt.float32
bf = mybir.dt.bfloat16

T, B, D, H = 512, 64, 512, 512
NCORES = 8
BS = B // 4          # 16 batch per core
G = 4 * H            # 2048 gates
GT = G // 128        # 16 gate tiles
KH = H // 128        # 4 k-chunks
HB = KH * BS         # 64 state cols
SC = 8               # scan steps per DMA batch
TC = 32              # proj timesteps per chunk
NCH = T // TC

SIG = mybir.ActivationFunctionType.Sigmoid
TANH = mybir.ActivationFunctionType.Tanh
IDENT = mybir.ActivationFunctionType.Identity

_compiled = {}


def _build(t_steps=T, dbg=False, nocc=False):
    nc = bacc.Bacc("TRN2", target_bir_lowering=False, debug=False,
                   num_devices=NCORES)
    nchunks = t_steps // TC

    xT = nc.dram_tensor("xT", (D, t_steps, BS), bf, kind="ExternalInput")
    maskd = nc.dram_tensor("maskd", (t_steps + 1, 128, HB), bf, kind="ExternalInput")
    whh0T = nc.dram_tensor("whh0T", (KH, 128, G), bf, kind="ExternalInput")
    wih0T = nc.dram_tensor("wih0T", (KH, 128, G), bf, kind="ExternalInput")
    whh1T = nc.dram_tensor("whh1T", (KH, 128, G), bf, kind="ExternalInput")
    wih1oT = nc.dram_tensor("wih1oT", (KH, 128, G), bf, kind="ExternalInput")
    wih1pT = nc.dram_tensor("wih1pT", (KH, 128, G), bf, kind="ExternalInput")
    identT = nc.dram_tensor("identT", (128, 128), bf, kind="ExternalInput")
    b0c = nc.dram_tensor("b0c", (GT, 128), f32, kind="ExternalInput")
    b1c = nc.dram_tensor("b1c", (GT, 128), f32, kind="ExternalInput")
    y1 = nc.dram_tensor("y1", (t_steps, 128, HB), bf, kind="ExternalOutput")
    if dbg:
        xwb0o = nc.dram_tensor("xwb0o", (t_steps, 128, GT * BS), bf, kind="ExternalOutput")
        y0o = nc.dram_tensor("y0o", (t_steps, 128, HB), bf, kind="ExternalOutput")
        xwb1o = nc.dram_tensor("xwb1o", (t_steps, 128, GT * BS), bf, kind="ExternalOutput")
    with tile.TileContext(nc) as tc:
        with (
            tc.tile_pool(name="wpool", bufs=1) as wpool,
            tc.tile_pool(name="xpool", bufs=3) as xpool,
            tc.tile_pool(name="gpool", bufs=3) as gpool,
            tc.tile_pool(name="spool", bufs=3) as spool,
            tc.tile_pool(name="opool", bufs=2) as opool,
            tc.tile_pool(name="mpool", bufs=2) as mpool,
            tc.tile_pool(name="state", bufs=1) as state,
            tc.tile_pool(name="psA", bufs=2, space="PSUM") as psA,
            tc.tile_pool(name="psS", bufs=1, space="PSUM") as psS,
            tc.tile_pool(name="dram", bufs=1, space="DRAM") as dram,
        ):
            xwb0 = dram.tile([t_steps, 128, GT * BS], bf)
            xwb1 = dram.tile([t_steps, 128, GT * BS], bf)
            y0 = dram.tile([t_steps, 128, HB], bf)
            ag = dram.tile([2 * t_steps, 128, HB], bf)

            def load_w(name, src):
                t = wpool.tile([128, KH * G], bf, tag=name)
                for k in range(KH):
                    nc.sync.dma_start(t[:, k * G:(k + 1) * G], src.ap()[k])
                return t

            whh0_sb = load_w("whh0", whh0T)
            wih0_sb = load_w("wih0", wih0T)
            whh1_sb = load_w("whh1", whh1T)
            wih1o_sb = load_w("wih1o", wih1oT)
            wih1p_sb = load_w("wih1p", wih1pT)
            ident_sb = wpool.tile([128, 128], bf, tag="ident")
            nc.sync.dma_start(ident_sb[:], identT.ap())
            bias_sb = wpool.tile([128, 2 * GT], f32, tag="bias")
            nc.sync.dma_start(bias_sb[:, 0:GT], b0c.ap().transpose([1, 0]))
            nc.sync.dma_start(bias_sb[:, GT:2 * GT], b1c.ap().transpose([1, 0]))

            # ---- input projections -> xwb dram ----
            # Emitted as a list of small "quanta" (thunks) so chunks beyond
            # the first two can be interleaved into the recurrent scan's PE
            # bubbles (the scan waits ~1us per step on the h-chain; proj
            # matmuls have no h dependency and fill that idle time).
            def proj_quanta(dst, w_sbs, srcs, bias_col, chunks):
                nk = len(w_sbs) * KH
                quanta = []
                for ch in chunks:
                    t0 = ch * TC
                    state = {}

                    def dma_q(t0=t0, state=state):
                        rhs = xpool.tile([128, nk, TC, BS], bf, tag="projx",
                                         name="projx")
                        ji = 0
                        for w_sb, src in zip(w_sbs, srcs):
                            for k in range(KH):
                                nc.sync.dma_start(rhs[:, ji], src(k, t0))
                                ji += 1
                        state["rhs"] = rhs
                    quanta.append(dma_q)

                    for g in range(GT):
                        halves = ([range(0, nk)] if nk <= 4 else
                                  [range(0, 4), range(4, nk)])
                        for hi, js in enumerate(halves):
                            def gate_q(t0=t0, g=g, js=js, hi=hi, nh=len(halves),
                                       state=state):
                                if hi == 0:
                                    state["ps"] = psA.tile(
                                        [128, TC * BS], f32, tag="psA",
                                        name="psA")
                                ps = state["ps"]
                                rhs = state["rhs"]
                                for ji in js:
                                    w_sb = w_sbs[ji // KH]
                                    k = ji % KH
                                    nc.tensor.matmul(
                                        ps[:],
                                        w_sb[:, k * G + g * 128: k * G + (g + 1) * 128],
                                        rhs[:, ji],
                                        start=(ji == 0),
                                        stop=(ji == nk - 1),
                                    )
                                if hi == nh - 1:
                                    g_sb = gpool.tile([128, TC * BS], bf,
                                                      tag="projg", name="projg")
                                    nc.scalar.activation(
                                        g_sb[:], ps[:], IDENT,
                                        bias=bias_sb[:, bias_col + g: bias_col + g + 1],
                                    )
                                    nc.sync.dma_start(
                                        dst[t0:t0 + TC, :, g * BS:(g + 1) * BS].transpose([1, 0, 2]),
                                        g_sb[:].rearrange("p (t b) -> p t b", t=TC),
                                    )
                            quanta.append(gate_q)
                return quanta

            def emit_all(quanta):
                for q in quanta:
                    q()

            srcs0 = [lambda k, t0: xT.ap()[k * 128:(k + 1) * 128, t0:t0 + TC, :]]
            emit_all(proj_quanta(xwb0, [wih0_sb], srcs0, 0, range(min(2, nchunks))))
            projA_rest = proj_quanta(xwb0, [wih0_sb], srcs0, 0,
                                     range(2, nchunks))

            # ---- recurrent scan ----
            # natural gate-type order in xw columns / weight columns: i,f,g,o
            QI = {"i": 0, "f": 1, "g": 2, "o": 3}

            def scan(xwb, whh_sb, y_dst, extra=()):
                h0 = state.tile([128, HB], bf, tag="h0")
                cst = state.tile([128, HB], f32, tag="cst")
                nc.gpsimd.memset(h0[:], 0.0)
                nc.gpsimd.memset(cst[:], 0.0)
                hprev = h0[:]
                xw = mk = ob = None
                extra = list(extra)
                nex = len(extra)
                spread = max(1, t_steps - 2 * TC)
                for t in range(t_steps):
                    j = t % SC
                    if j == 0:
                        xw = xpool.tile([128, SC, GT * BS], bf, tag="scanx")
                        nc.sync.dma_start(xw[:], xwb[t:t + SC].transpose([1, 0, 2]))
                        mk = mpool.tile([128, SC + 1, HB], bf, tag="mk")
                        nc.sync.dma_start(
                            mk[:], maskd.ap()[t:t + SC + 1].transpose([1, 0, 2]))
                        ob = opool.tile([128, SC, HB], bf, tag="ob")
                    # c state mask (c *= m[t]); h mask was folded into hprev
                    if t > 0:
                        nc.vector.tensor_mul(cst[:], cst[:], mk[:, j])
                    ps = {q: psS.tile([128, 512], f32, tag=f"ps{q}",
                                      name=f"ps{q}") for q in "ifgo"}
                    for q in "ifgo":
                        nc.tensor.matmul(
                            ps[q][:, 0:HB], ident_sb[:],
                            xw[:, j, QI[q] * HB:(QI[q] + 1) * HB],
                            start=True, stop=False, skip_group_check=True)
                    for q in "gifo":
                        qi = QI[q]
                        for gt in range(4):
                            for k in range(KH):
                                nc.tensor.matmul(
                                    ps[q][:, gt * BS:(gt + 1) * BS],
                                    whh_sb[:, k * G + qi * 512 + gt * 128:
                                           k * G + qi * 512 + (gt + 1) * 128],
                                    hprev[:, k * BS:(k + 1) * BS],
                                    start=False, stop=(k == KH - 1),
                                    skip_group_check=True)
                    tg = spool.tile([128, HB], f32, tag="tg")
                    nc.scalar.activation(tg[:], ps["g"][:, 0:HB], TANH)
                    si = spool.tile([128, HB], f32, tag="si")
                    nc.scalar.activation(si[:], ps["i"][:, 0:HB], SIG)
                    ig = spool.tile([128, HB], f32, tag="ig")
                    nc.vector.tensor_mul(ig[:], si[:], tg[:])
                    sf = spool.tile([128, HB], f32, tag="sf")
                    nc.scalar.activation(sf[:], ps["f"][:, 0:HB], SIG)
                    fc = spool.tile([128, HB], f32, tag="fc")
                    nc.vector.tensor_mul(fc[:], sf[:], cst[:])
                    nc.vector.tensor_add(cst[:], fc[:], ig[:])
                    so = spool.tile([128, HB], f32, tag="so")
                    nc.scalar.activation(so[:], ps["o"][:, 0:HB], SIG)
                    tc2 = spool.tile([128, HB], f32, tag="tc2")
                    nc.scalar.activation(tc2[:], cst[:], TANH)
                    if t + 1 < t_steps:
                        # next-state path first: hm = (so*m[t+1]) * tanh(c)
                        som = spool.tile([128, HB], f32, tag="som")
                        nc.vector.tensor_mul(som[:], so[:], mk[:, j + 1])
                        hm = spool.tile([128, HB], bf, tag="hm")
                        nc.vector.tensor_mul(hm[:], som[:], tc2[:])
                        hprev = hm[:]
                    h2 = spool.tile([128, HB], f32, tag="h2")
                    nc.vector.tensor_mul(h2[:], so[:], tc2[:])
                    nc.vector.tensor_mul(ob[:, j], h2[:], mk[:, j])
                    if j == SC - 1:
                        nc.sync.dma_start(
                            y_dst[t - SC + 1:t + 1].transpose([1, 0, 2]), ob[:])
                    # fill this step's PE bubble with interleaved proj work
                    q0 = nex * t // spread
                    q1 = nex * (t + 1) // spread
                    for q in extra[q0:min(q1, nex)]:
                        q()

            scan(xwb0, whh0_sb, y0, extra=projA_rest)
            if dbg:
                nc.sync.dma_start(xwb0o.ap()[:], xwb0[:])
                nc.sync.dma_start(y0o.ap()[:], y0[:])

            # ---- exchange (pairwise fwd<->bwd) ----
            if nocc:
                nc.sync.dma_start(ag[0:t_steps], y0[:])
                nc.sync.dma_start(ag[t_steps:2 * t_steps], y0[:])
                partner_row = nc.snap(t_steps)
            else:
                nc.gpsimd.collective_compute(
                    "AllGather", mybir.AluOpType.bypass,
                    ins=[y0.opt()], outs=[ag.opt()],
                    replica_groups=[[0, 4], [1, 5], [2, 6], [3, 7]],
                )
                partner_row = nc.snap(
                    ((nc.partition_id() // 4 + 1) % 2) * t_steps)

            def par_src(k, t0):
                # partner rows, time-reversed: own step tau needs partner row
                # (T-1-tau); rows [T-TC-t0, T-t0) reversed.
                return (ag[bass.ds(partner_row + (t_steps - TC - t0), TC)]
                        [::-1, :, k * BS:(k + 1) * BS].transpose([1, 0, 2]))

            srcs1 = [
                lambda k, t0: y0[t0:t0 + TC, :, k * BS:(k + 1) * BS].transpose([1, 0, 2]),
                par_src,
            ]
            emit_all(proj_quanta(xwb1, [wih1o_sb, wih1p_sb], srcs1, GT,
                                 range(min(2, nchunks))))
            projD_rest = proj_quanta(xwb1, [wih1o_sb, wih1p_sb], srcs1, GT,
                                     range(2, nchunks))

            scan(xwb1, whh1_sb, y1.ap(), extra=projD_rest)
            if dbg:
                nc.sync.dma_start(xwb1o.ap()[:], xwb1[:])

    nc.compile()
    return nc


def _prep_inputs(x, lengths, weights, t_steps=T):
    active = (np.arange(T)[:, None] < np.asarray(lengths)[None, :]).astype(np.float32)
    ident = np.eye(128, dtype=bf16)
    in_maps = []
    for c in range(NCORES):
        d, s = c // 4, c % 4
        bsl = slice(s * BS, (s + 1) * BS)
        pre = "f" if d == 0 else "b"
        xs = np.asarray(x[:, bsl, :], np.float32)
        am = active[:, bsl]
        if d == 1:
            xs = xs[::-1]
            am = am[::-1]
        xs = xs[:t_steps]
        am = am[:t_steps]

        W_ih0 = np.asarray(weights[f"{pre}W_ih0"], np.float32)
        W_hh0 = np.asarray(weights[f"{pre}W_hh0"], np.float32)
        W_ih1 = np.asarray(weights[f"{pre}W_ih1"], np.float32)
        W_hh1 = np.asarray(weights[f"{pre}W_hh1"], np.float32)
        b0 = np.asarray(weights[f"{pre}b0"], np.float32)
        b1 = np.asarray(weights[f"{pre}b1"], np.float32)
        own = W_ih1[:, :512] if d == 0 else W_ih1[:, 512:]
        par = W_ih1[:, 512:] if d == 0 else W_ih1[:, :512]

        amk = np.tile(am, (1, KH)).astype(bf16)          # [T, HB]
        mfull = np.zeros((t_steps + 1, 128, HB), bf16)
        mfull[:t_steps] = amk[:, None, :]

        in_maps.append({
            "xT": np.ascontiguousarray(xs.transpose(2, 0, 1)).astype(bf16),
            "maskd": mfull,
            "whh0T": np.ascontiguousarray(W_hh0.T.reshape(KH, 128, G)).astype(bf16),
            "wih0T": np.ascontiguousarray(W_ih0.T.reshape(KH, 128, G)).astype(bf16),
            "whh1T": np.ascontiguousarray(W_hh1.T.reshape(KH, 128, G)).astype(bf16),
            "wih1oT": np.ascontiguousarray(own.T.reshape(KH, 128, G)).astype(bf16),
            "wih1pT": np.ascontiguousarray(par.T.reshape(KH, 128, G)).astype(bf16),
            "identT": ident,
            "b0c": np.ascontiguousarray(b0.reshape(GT, 128)).astype(np.float32),
            "b1c": np.ascontiguousarray(b1.reshape(GT, 128)).astype(np.float32),
        })
    return in_maps


def _assemble(results, t_steps=T):
    out = np.zeros((t_steps, B, 2 * H), np.float32)
    for c in range(NCORES):
        d, s = c // 4, c % 4
        arr = results[c]["y1"].astype(np.float32).reshape(t_steps, 128, KH, BS)
        if d == 1:
            arr = arr[::-1]
        blk = arr.transpose(0, 3, 2, 1).reshape(t_steps, BS, H)
        out[:, s * BS:(s + 1) * BS, d * H:(d + 1) * H] = blk
    return out


def kernel(x, lengths, fW_ih0, fW_hh0, fb0, bW_ih0, bW_hh0, bb0,
           fW_ih1, fW_hh1, fb1, bW_ih1, bW_hh1, bb1, _t_steps=T,
           _want_trace=False, _dbg=False):
    weights = dict(fW_ih0=fW_ih0, fW_hh0=fW_hh0, fb0=fb0,
                   bW_ih0=bW_ih0, bW_hh0=bW_hh0, bb0=bb0,
                   fW_ih1=fW_ih1, fW_hh1=fW_hh1, fb1=fb1,
                   bW_ih1=bW_ih1, bW_hh1=bW_hh1, bb1=bb1)
    key = (_t_steps, _dbg)
    if key not in _compiled:
        _compiled[key] = _build(_t_steps, dbg=_dbg)
    nc = _compiled[key]
    in_maps = _prep_inputs(x, lengths, weights, _t_steps)
    res = bass_utils.run_bass_kernel_spmd(
        nc, in_maps, core_ids=list(range(NCORES)), trace=_want_trace)
    out = _assemble(res.results, _t_steps)
    if _want_trace or _dbg:
        kernel.last_results = res
    return out
